# revision 2
# baseline (speedup 1.0000x reference)
"""Trainium2 Bass kernel for nn_CelestialWaveAggregator.

Math: out[b,s,c] = tanh(h_c(agg[b,s,c])) where agg = wave_features @ M.T (M is
the per-body softmax aggregation matrix over ragged wave groups) and h_c is the
per-body 1->32->64->32->1 gelu MLP collapsed to a *univariate* function of the
aggregated scalar.

v2 device strategy (8 cores, batch-sharded 2048*8 rows/core):
  - All device data in fp16: input DMA bytes halved (one big 16KB-per-row
    descriptor DMA per chunk), PE matmul at full 1 col/cycle rate.
  - PE: agg matmuls accumulate a [104, 1024] PSUM tile from 8 replica-masked
    fp16 weight blocks (8 row-chunks x 13 bodies on partitions), r-outer so
    consecutive matmuls share the stationary weights.
  - ACT: t = Identity(ps + bias) -> fp16; u = Square(t); final tanh with
    per-partition scale a_D.
  - DVE: degree-10 polynomial evaluated as a product of 5 quadratics
    p(t) = a_D * prod_i (t^2 + alpha_i t + beta_i).  Each quadratic is one
    tensor_scalar (4x fp16 fast mode) + one tensor_tensor add (2x), and the
    product tree is 4 tensor_tensor mults (2x) — avoiding scalar_tensor_tensor
    which has NO fp16 fast mode.
Output is stored feature-major [104, 2048] fp16 per core; the host permutes to
row-major f32 during the gather/unshard step.  Roots/coefficients are fit on
host from the (tiny) MLP weights; quadratics are interleaved big/small so fp16
partial products stay in range.  End-to-end host-simulated rel err ~3.9e-3.
"""

import math

import numpy as np

# ---- problem constants (hardcoded per contract) ----
LENS = np.array([9, 9, 9, 9, 9, 9, 9, 9, 9, 9, 12, 8, 3])
STARTS = np.concatenate([[5], 5 + np.cumsum(LENS)[:-1]])
MAXW, NW, NB = 12, 118, 13
B, S = 32, 4096
NCORES = 8
RPC = (B * S) // NCORES          # 16384 rows per core
NREP = 8                         # replica groups on partitions (8*13=104)
NP_USED = NREP * NB              # 104 used partitions
F = RPC // NREP                  # 2048 free columns per partition (exact)
NCHUNK = 2
W = F // NCHUNK                  # 1024 columns per chunk per replica
SUPER = NREP * W                 # 8192 dram columns per chunk DMA
DEG = 10                         # polynomial degree (even)
NQ = DEG // 2                    # quadratic factors
NCONST = 2 + 2 * NQ              # nmid, alphas, betas, aD

_f64 = np.float64


def _erf(x):
    try:
        from scipy.special import erf
        return erf(x)
    except Exception:
        return np.vectorize(math.erf)(x)


def _gelu(x):
    return 0.5 * x * (1.0 + _erf(x / np.sqrt(2.0)))


def _build_M(agg_logits):
    """Dense [13, 118] aggregation matrix from ragged softmax groups."""
    al = np.asarray(agg_logits, _f64)
    valid = np.arange(MAXW)[None, :] < LENS[:, None]
    logits = np.where(valid, al, -np.inf)
    w = np.exp(logits - logits.max(axis=-1, keepdims=True))
    w = w / w.sum(axis=-1, keepdims=True)
    w = np.where(valid, w, 0.0)
    M = np.zeros((NB, NW))
    idx = np.clip(STARTS[:, None] + np.arange(MAXW)[None, :], 0, NW - 1)
    for c in range(NB):
        for j in range(MAXW):
            M[c, idx[c, j]] += w[c, j]
    return M


def _h_fn(x, c, W1, b1, W2, b2, W3, b3, W4, b4):
    """Pre-tanh univariate MLP for body c, float64."""
    a = x[:, None] * W1[c, 0][None, :] + b1[c]
    h1 = _gelu(a)
    h2 = _gelu(h1 @ W2[c] + b2[c])
    h3 = _gelu(h2 @ W3[c] + b3[c])
    return h3 @ W4[c][:, 0] + b4[c, 0]


def _fit_tables(inputs):
    """Host precompute: aggregation matrix, per-body quadratic factorizations.

    Returns (Wm fp16 [118, 8*104], consts f32 [104, NCONST]).
    consts cols: 0 = -mid*invhalf; 1..NQ = alpha_i; NQ+1..2NQ = beta_i;
    2NQ+1 = a_D (leading coeff, applied as tanh scale)."""
    M = _build_M(inputs["agg_logits"])
    Wmlp = {k: np.asarray(inputs[k], _f64) for k in
            ("W1", "b1", "W2", "b2", "W3", "b3", "W4", "b4")}

    # calibration: per-body agg range from the actual data (+ margin)
    X = np.asarray(inputs["wave_features"], np.float32).reshape(-1, NW)
    agg = X.astype(_f64) @ M.T
    lo = agg.min(axis=0)
    hi = agg.max(axis=0)
    m = 0.12 * (hi - lo)
    lo, hi = lo - m, hi + m
    mid = 0.5 * (lo + hi)
    invhalf = 2.0 / (hi - lo)

    quads = np.zeros((NB, NQ, 2))
    aD = np.zeros(NB)
    tg2 = np.linspace(-1, 1, 513)
    for c in range(NB):
        xs = np.linspace(lo[c], hi[c], 3001)
        hs = _h_fn(xs, c, **Wmlp)
        ys = np.tanh(hs)
        t = (xs - mid[c]) * invhalf[c]
        V = np.polynomial.chebyshev.chebvander(t, DEG)
        wgt = 1.0 / np.cosh(hs) ** 2 + 1e-4
        for _ in range(10):
            sw = np.sqrt(wgt)
            coef, *_r = np.linalg.lstsq(V * sw[:, None], hs * sw, rcond=None)
            err = np.abs(np.tanh(V @ coef) - ys)
            wgt = wgt * (1.0 + 1.5 * err / (err.max() + 1e-12))
        a = np.polynomial.chebyshev.cheb2poly(coef)     # power basis a[0..DEG]
        aD[c] = a[DEG]
        roots = np.roots(a[::-1])
        # pair complex-conjugate roots; pair real roots sorted-adjacent
        cplx = [r for r in roots if abs(r.imag) > 1e-9]
        real = sorted(r.real for r in roots if abs(r.imag) <= 1e-9)
        qs = []
        used = set()
        for i, r in enumerate(cplx):
            if i in used:
                continue
            for j2 in range(i + 1, len(cplx)):
                if j2 not in used and abs(cplx[j2].conjugate() - r) < \
                        1e-6 * max(1.0, abs(r)):
                    used.add(i)
                    used.add(j2)
                    qs.append((-2 * r.real, abs(r) ** 2))
                    break
        for i in range(0, len(real), 2):
            qs.append((-(real[i] + real[i + 1]), real[i] * real[i + 1]))
        assert len(qs) == NQ, f"body {c}: got {len(qs)} quadratic factors"
        # interleave large/small |q|max so fp16 partial products stay in range
        mx = [np.abs(tg2 ** 2 + al * tg2 + be).max() for al, be in qs]
        order = list(np.argsort(mx)[::-1])
        inter = []
        lo_i, hi_i = 0, len(order) - 1
        for k2 in range(len(order)):
            inter.append(order[hi_i] if k2 % 2 == 0 else order[lo_i])
            if k2 % 2 == 0:
                hi_i -= 1
            else:
                lo_i += 1
        qs = [qs[i] for i in inter]
        quads[c] = qs
        P = np.ones_like(tg2)
        for al, be in qs:
            P = P * (tg2 ** 2 + al * tg2 + be)
            assert np.abs(P).max() < 3e4, f"body {c}: fp16 overflow risk"

    # device constant tensors
    Wm = np.zeros((NW, NREP * NP_USED), np.float16)
    Ms = (M * invhalf[:, None]).T.astype(np.float32)  # [118, 13]
    for r in range(NREP):
        for c in range(NB):
            Wm[:, r * NP_USED + r * NB + c] = Ms[:, c].astype(np.float16)
    consts = np.zeros((NP_USED, NCONST), np.float32)
    for r in range(NREP):
        for c in range(NB):
            q = r * NB + c
            consts[q, 0] = -mid[c] * invhalf[c]
            for i in range(NQ):
                consts[q, 1 + i] = quads[c, i, 0]
                consts[q, 1 + NQ + i] = quads[c, i, 1]
            consts[q, 1 + 2 * NQ] = aD[c]
    return Wm, consts


_PROGRAM = None


def _build_program():
    """Build + compile the (SPMD, per-core) Bass/Tile program once."""
    global _PROGRAM
    if _PROGRAM is not None:
        return _PROGRAM

    from contextlib import ExitStack
    import concourse.bacc as bacc
    import concourse.tile as tile
    import concourse.mybir as mybir
    from concourse._compat import axon_active

    f32 = mybir.dt.float32
    f16 = mybir.dt.float16
    Alu = mybir.AluOpType
    Act = mybir.ActivationFunctionType

    nc = bacc.Bacc(
        "TRN2",
        target_bir_lowering=False,
        debug=not axon_active(),
        enable_asserts=True,
        num_devices=NCORES,
    )
    xt = nc.dram_tensor("xt", [NW, RPC], f16, kind="ExternalInput").ap()
    wm = nc.dram_tensor("wm", [NW, NREP * NP_USED], f16, kind="ExternalInput").ap()
    cst = nc.dram_tensor("cst", [NP_USED, NCONST], f32, kind="ExternalInput").ap()
    out = nc.dram_tensor("out", [NP_USED, F], f16, kind="ExternalOutput").ap()

    with tile.TileContext(nc) as tc, ExitStack() as ctx:
        cpool = ctx.enter_context(tc.tile_pool(name="consts", bufs=1))
        xpool = ctx.enter_context(tc.tile_pool(name="xin", bufs=2))
        ppool = ctx.enter_context(tc.tile_pool(name="ps", bufs=2, space="PSUM"))
        tpool = ctx.enter_context(tc.tile_pool(name="tt", bufs=2))
        upool = ctx.enter_context(tc.tile_pool(name="uu", bufs=2))
        qpool = ctx.enter_context(tc.tile_pool(name="qq", bufs=2 * NQ))
        ypool = ctx.enter_context(tc.tile_pool(name="yy", bufs=2))

        wm_sb = cpool.tile([NW, NREP * NP_USED], f16)
        nc.gpsimd.dma_start(wm_sb[:], wm[:])
        cst_sb = cpool.tile([NP_USED, NCONST], f32)
        nc.gpsimd.dma_start(cst_sb[:], cst[:])

        for j in range(NCHUNK):
            xc = xpool.tile([NW, SUPER], f16, tag="xin")
            nc.gpsimd.dma_start(xc[:], xt[:, j * SUPER:(j + 1) * SUPER])
            ps = ppool.tile([NP_USED, W], f32, tag="ps")
            for r in range(NREP):
                for h0 in range(0, W, 512):
                    nc.tensor.matmul(
                        ps[:, h0:h0 + 512],
                        wm_sb[:, r * NP_USED:(r + 1) * NP_USED],
                        xc[:, r * W + h0: r * W + h0 + 512],
                        start=(r == 0),
                        stop=(r == NREP - 1),
                    )
            # t = agg*invhalf - mid*invhalf, fp16; u = t^2
            t16 = tpool.tile([NP_USED, W], f16, tag="tt")
            nc.scalar.activation(t16[:], ps[:], Act.Identity,
                                 bias=cst_sb[:, 0:1])
            u16 = upool.tile([NP_USED, W], f16, tag="uu")
            nc.scalar.activation(u16[:], t16[:], Act.Square)
            # quadratic factors q_i = u + alpha_i t + beta_i
            qs = []
            for i in range(NQ):
                q = qpool.tile([NP_USED, W], f16, tag=f"q{i}")
                nc.vector.tensor_scalar(
                    q[:], t16[:], cst_sb[:, 1 + i:2 + i],
                    cst_sb[:, 1 + NQ + i:2 + NQ + i],
                    op0=Alu.mult, op1=Alu.add)
                nc.vector.tensor_tensor(q[:], q[:], u16[:], op=Alu.add)
                qs.append(q)
            # product tree (in-place into q0/q2)
            nc.vector.tensor_tensor(qs[0][:], qs[0][:], qs[1][:], op=Alu.mult)
            nc.vector.tensor_tensor(qs[2][:], qs[2][:], qs[3][:], op=Alu.mult)
            nc.vector.tensor_tensor(qs[0][:], qs[0][:], qs[2][:], op=Alu.mult)
            if NQ == 5:
                nc.vector.tensor_tensor(qs[0][:], qs[0][:], qs[4][:],
                                        op=Alu.mult)
            # y = tanh(a_D * prod)
            y16 = ypool.tile([NP_USED, W], f16, tag="yy")
            nc.scalar.activation(y16[:], qs[0][:], Act.Tanh,
                                 scale=cst_sb[:, 1 + 2 * NQ:2 + 2 * NQ])
            nc.sync.dma_start(out[:, j * W:(j + 1) * W], y16[:])

    nc.compile()
    _PROGRAM = nc
    return nc


LAST_EXEC_NS = None


def kernel(**inputs) -> np.ndarray:
    global LAST_EXEC_NS
    import os
    from concourse.bass_utils import run_bass_kernel_spmd

    Wm, consts = _fit_tables(inputs)
    X = np.ascontiguousarray(
        np.asarray(inputs["wave_features"], np.float32).reshape(B * S, NW)
    )

    in_maps = []
    for k in range(NCORES):
        XT = X[k * RPC:(k + 1) * RPC].T  # [118, 16384], col = r*F + f
        # chunk-concatenated layout: col = j*SUPER + r*W + (f - j*W)
        xt_k = np.ascontiguousarray(
            XT.reshape(NW, NREP, NCHUNK, W).transpose(0, 2, 1, 3)
            .reshape(NW, RPC).astype(np.float16))
        in_maps.append({"xt": xt_k, "wm": Wm, "cst": consts})

    nc = _build_program()
    trace = os.environ.get("BASS_KERNEL_PROFILE") == "1"
    res = run_bass_kernel_spmd(nc, in_maps, core_ids=list(range(NCORES)),
                               trace=trace)
    LAST_EXEC_NS = res.exec_time_ns
    # unshard: [104, 2048] fp16 feature-major -> [16384, 13] f32 row-major
    outs = []
    for k in range(NCORES):
        buf = np.asarray(res.results[k]["out"], np.float16).astype(np.float32)
        outs.append(buf.reshape(NREP, NB, F).transpose(0, 2, 1).reshape(RPC, NB))
    return np.concatenate(outs, axis=0).reshape(B, S, NB)


# revision 3
# speedup vs baseline: 1.6067x; 1.6067x over previous
"""Trainium2 Bass kernel for nn_CelestialWaveAggregator.

Math: out[b,s,c] = tanh(h_c(agg[b,s,c])) where agg = wave_features @ M.T (M is
the per-body softmax aggregation matrix over ragged wave groups) and h_c is the
per-body 1->32->64->32->1 gelu MLP collapsed to a *univariate* function of the
aggregated scalar.

v3 device strategy (8 cores, batch-sharded 2048*8 rows/core):
  - bf16 inputs: half the DMA bytes of f32; PE matmul streams at the 16-bit
    rate.  Input DMA is issued as 4 row-slice dma_starts per chunk (each
    SWDGE dma_start only stripes over ~2 of the 16 DMA engines, so >=8
    concurrent instructions are needed to engage the full per-core HBM BW).
  - PE: agg matmuls accumulate [104, W] PSUM tiles from 8 replica-masked bf16
    weight blocks (8 row-chunks x 13 bodies on partitions).
  - ACT: t = Identity(ps + bias) -> fp16; u = Square(t); final tanh with
    per-partition scale a_D.
  - DVE: degree-8 polynomial evaluated as a product of 4 quadratics
    p(t) = a_D * prod_i (t^2 + alpha_i t + beta_i).  Each quadratic is one
    tensor_scalar (4x fp16 fast mode) + one tensor_tensor add (2x), and the
    product tree is 3 tensor_tensor mults (2x) — avoiding scalar_tensor_tensor
    which has NO fp16 fast mode.
  - 3 chunks [512, 768, 768] so the first PSUM/poly work starts early while
    later input still streams.
Output is stored feature-major [104, 2048] fp16 per core (split row-wise
across 2 store DMAs per chunk); host permutes to row-major f32 on gather.
Roots/coefficients are fit on host from the (tiny) MLP weights; quadratics are
interleaved big/small so fp16 partial products stay in range.  Host-simulated
end-to-end rel err ~9.4e-3 (gate 2e-2).
"""

import math

import numpy as np

# ---- problem constants (hardcoded per contract) ----
LENS = np.array([9, 9, 9, 9, 9, 9, 9, 9, 9, 9, 12, 8, 3])
STARTS = np.concatenate([[5], 5 + np.cumsum(LENS)[:-1]])
MAXW, NW, NB = 12, 118, 13
B, S = 32, 4096
NCORES = 8
RPC = (B * S) // NCORES          # 16384 rows per core
NREP = 8                         # replica groups on partitions (8*13=104)
NP_USED = NREP * NB              # 104 used partitions
F = RPC // NREP                  # 2048 free columns per partition (exact)
CHUNKS = [512, 768, 768]         # pipeline chunk widths (sum = F)
DEG = 8                          # polynomial degree (even)
NQ = DEG // 2                    # quadratic factors
NCONST = 2 + 2 * NQ              # nmid, alphas, betas, aD
ROWSLICES = [(0, 30), (30, 60), (60, 90), (90, 118)]   # input DMA row split
OUTSLICES = [(0, 52), (52, 104)]                       # output DMA row split

_f64 = np.float64


def _bf16_dtype():
    import ml_dtypes
    return ml_dtypes.bfloat16


def _erf(x):
    try:
        from scipy.special import erf
        return erf(x)
    except Exception:
        return np.vectorize(math.erf)(x)


def _gelu(x):
    return 0.5 * x * (1.0 + _erf(x / np.sqrt(2.0)))


def _build_M(agg_logits):
    """Dense [13, 118] aggregation matrix from ragged softmax groups."""
    al = np.asarray(agg_logits, _f64)
    valid = np.arange(MAXW)[None, :] < LENS[:, None]
    logits = np.where(valid, al, -np.inf)
    w = np.exp(logits - logits.max(axis=-1, keepdims=True))
    w = w / w.sum(axis=-1, keepdims=True)
    w = np.where(valid, w, 0.0)
    M = np.zeros((NB, NW))
    idx = np.clip(STARTS[:, None] + np.arange(MAXW)[None, :], 0, NW - 1)
    for c in range(NB):
        for j in range(MAXW):
            M[c, idx[c, j]] += w[c, j]
    return M


def _h_fn(x, c, W1, b1, W2, b2, W3, b3, W4, b4):
    """Pre-tanh univariate MLP for body c, float64."""
    a = x[:, None] * W1[c, 0][None, :] + b1[c]
    h1 = _gelu(a)
    h2 = _gelu(h1 @ W2[c] + b2[c])
    h3 = _gelu(h2 @ W3[c] + b3[c])
    return h3 @ W4[c][:, 0] + b4[c, 0]


def _fit_tables(inputs):
    """Host precompute: aggregation matrix, per-body quadratic factorizations.

    Returns (Wm bf16 [118, 8*104], consts f32 [104, NCONST]).
    consts cols: 0 = -mid*invhalf; 1..NQ = alpha_i; NQ+1..2NQ = beta_i;
    2NQ+1 = a_D (leading coeff, applied as tanh scale)."""
    M = _build_M(inputs["agg_logits"])
    Wmlp = {k: np.asarray(inputs[k], _f64) for k in
            ("W1", "b1", "W2", "b2", "W3", "b3", "W4", "b4")}

    # calibration: per-body agg range from the actual data (+ margin)
    X = np.asarray(inputs["wave_features"], np.float32).reshape(-1, NW)
    agg = X.astype(_f64) @ M.T
    lo = agg.min(axis=0)
    hi = agg.max(axis=0)
    m = 0.12 * (hi - lo)
    lo, hi = lo - m, hi + m
    mid = 0.5 * (lo + hi)
    invhalf = 2.0 / (hi - lo)

    quads = np.zeros((NB, NQ, 2))
    aD = np.zeros(NB)
    tg2 = np.linspace(-1, 1, 513)
    for c in range(NB):
        xs = np.linspace(lo[c], hi[c], 3001)
        hs = _h_fn(xs, c, **Wmlp)
        ys = np.tanh(hs)
        t = (xs - mid[c]) * invhalf[c]
        V = np.polynomial.chebyshev.chebvander(t, DEG)
        wgt = 1.0 / np.cosh(hs) ** 2 + 1e-4
        for _ in range(10):
            sw = np.sqrt(wgt)
            coef, *_r = np.linalg.lstsq(V * sw[:, None], hs * sw, rcond=None)
            err = np.abs(np.tanh(V @ coef) - ys)
            wgt = wgt * (1.0 + 1.5 * err / (err.max() + 1e-12))
        a = np.polynomial.chebyshev.cheb2poly(coef)     # power basis a[0..DEG]
        aD[c] = a[DEG]
        roots = np.roots(a[::-1])
        # pair complex-conjugate roots; pair real roots sorted-adjacent
        cplx = [r for r in roots if abs(r.imag) > 1e-9]
        real = sorted(r.real for r in roots if abs(r.imag) <= 1e-9)
        qs = []
        used = set()
        for i, r in enumerate(cplx):
            if i in used:
                continue
            for j2 in range(i + 1, len(cplx)):
                if j2 not in used and abs(cplx[j2].conjugate() - r) < \
                        1e-6 * max(1.0, abs(r)):
                    used.add(i)
                    used.add(j2)
                    qs.append((-2 * r.real, abs(r) ** 2))
                    break
        for i in range(0, len(real), 2):
            qs.append((-(real[i] + real[i + 1]), real[i] * real[i + 1]))
        assert len(qs) == NQ, f"body {c}: got {len(qs)} quadratic factors"
        # interleave large/small |q|max so fp16 partial products stay in range
        mx = [np.abs(tg2 ** 2 + al * tg2 + be).max() for al, be in qs]
        order = list(np.argsort(mx)[::-1])
        inter = []
        lo_i, hi_i = 0, len(order) - 1
        for k2 in range(len(order)):
            inter.append(order[hi_i] if k2 % 2 == 0 else order[lo_i])
            if k2 % 2 == 0:
                hi_i -= 1
            else:
                lo_i += 1
        qs = [qs[i] for i in inter]
        quads[c] = qs
        P = np.ones_like(tg2)
        for al, be in qs:
            P = P * (tg2 ** 2 + al * tg2 + be)
            assert np.abs(P).max() < 3e4, f"body {c}: fp16 overflow risk"

    # device constant tensors
    bf16 = _bf16_dtype()
    Wm = np.zeros((NW, NREP * NP_USED), bf16)
    Ms = (M * invhalf[:, None]).T.astype(np.float32)  # [118, 13]
    for r in range(NREP):
        for c in range(NB):
            Wm[:, r * NP_USED + r * NB + c] = Ms[:, c].astype(bf16)
    consts = np.zeros((NP_USED, NCONST), np.float32)
    for r in range(NREP):
        for c in range(NB):
            q = r * NB + c
            consts[q, 0] = -mid[c] * invhalf[c]
            for i in range(NQ):
                consts[q, 1 + i] = quads[c, i, 0]
                consts[q, 1 + NQ + i] = quads[c, i, 1]
            consts[q, 1 + 2 * NQ] = aD[c]
    return Wm, consts


def _split512(w):
    """Split a chunk width into PSUM-bank-aligned matmul runs (<=512 each)."""
    out = []
    while w > 0:
        out.append(min(w, 512))
        w -= out[-1]
    return out


_PROGRAM = None


def _build_program():
    """Build + compile the (SPMD, per-core) Bass/Tile program once."""
    global _PROGRAM
    if _PROGRAM is not None:
        return _PROGRAM

    from contextlib import ExitStack
    import concourse.bacc as bacc
    import concourse.tile as tile
    import concourse.mybir as mybir
    from concourse._compat import axon_active

    f32 = mybir.dt.float32
    f16 = mybir.dt.float16
    bf16 = mybir.dt.bfloat16
    Alu = mybir.AluOpType
    Act = mybir.ActivationFunctionType

    nc = bacc.Bacc(
        "TRN2",
        target_bir_lowering=False,
        debug=not axon_active(),
        enable_asserts=True,
        num_devices=NCORES,
    )
    xt = nc.dram_tensor("xt", [NW, RPC], bf16, kind="ExternalInput").ap()
    wm = nc.dram_tensor("wm", [NW, NREP * NP_USED], bf16,
                        kind="ExternalInput").ap()
    cst = nc.dram_tensor("cst", [NP_USED, NCONST], f32, kind="ExternalInput").ap()
    out = nc.dram_tensor("out", [NP_USED, F], f16, kind="ExternalOutput").ap()

    with tile.TileContext(nc) as tc, ExitStack() as ctx:
        cpool = ctx.enter_context(tc.tile_pool(name="consts", bufs=1))
        xpool = ctx.enter_context(tc.tile_pool(name="xin", bufs=2))
        ppool = ctx.enter_context(tc.tile_pool(name="ps", bufs=2, space="PSUM"))
        tpool = ctx.enter_context(tc.tile_pool(name="tt", bufs=2))
        upool = ctx.enter_context(tc.tile_pool(name="uu", bufs=2))
        qpool = ctx.enter_context(tc.tile_pool(name="qq", bufs=2 * NQ))
        ypool = ctx.enter_context(tc.tile_pool(name="yy", bufs=2))

        wm_sb = cpool.tile([NW, NREP * NP_USED], bf16)
        nc.sync.dma_start(wm_sb[:], wm[:])
        cst_sb = cpool.tile([NP_USED, NCONST], f32)
        nc.sync.dma_start(cst_sb[:], cst[:])

        c_off = 0
        for j, W in enumerate(CHUNKS):
            SUP = NREP * W
            base = NREP * c_off
            xc = xpool.tile([NW, SUP], bf16, tag="xin")
            for (r0, r1) in ROWSLICES:
                nc.gpsimd.dma_start(xc[r0:r1, :], xt[r0:r1, base:base + SUP])
            ps = ppool.tile([NP_USED, W], f32, tag="ps")
            for r in range(NREP):
                h0 = 0
                for hw in _split512(W):
                    nc.tensor.matmul(
                        ps[:, h0:h0 + hw],
                        wm_sb[:, r * NP_USED:(r + 1) * NP_USED],
                        xc[:, r * W + h0: r * W + h0 + hw],
                        start=(r == 0),
                        stop=(r == NREP - 1),
                    )
                    h0 += hw
            # t = agg*invhalf - mid*invhalf, fp16; u = t^2
            t16 = tpool.tile([NP_USED, W], f16, tag="tt")
            nc.scalar.activation(t16[:], ps[:], Act.Identity,
                                 bias=cst_sb[:, 0:1])
            u16 = upool.tile([NP_USED, W], f16, tag="uu")
            nc.scalar.activation(u16[:], t16[:], Act.Square)
            # quadratic factors q_i = u + alpha_i t + beta_i
            qs = []
            for i in range(NQ):
                q = qpool.tile([NP_USED, W], f16, tag=f"q{i}")
                nc.vector.tensor_scalar(
                    q[:], t16[:], cst_sb[:, 1 + i:2 + i],
                    cst_sb[:, 1 + NQ + i:2 + NQ + i],
                    op0=Alu.mult, op1=Alu.add)
                nc.vector.tensor_tensor(q[:], q[:], u16[:], op=Alu.add)
                qs.append(q)
            # product tree (in-place into q0/q2)
            nc.vector.tensor_tensor(qs[0][:], qs[0][:], qs[1][:], op=Alu.mult)
            nc.vector.tensor_tensor(qs[2][:], qs[2][:], qs[3][:], op=Alu.mult)
            nc.vector.tensor_tensor(qs[0][:], qs[0][:], qs[2][:], op=Alu.mult)
            # y = tanh(a_D * prod)
            y16 = ypool.tile([NP_USED, W], f16, tag="yy")
            nc.scalar.activation(y16[:], qs[0][:], Act.Tanh,
                                 scale=cst_sb[:, 1 + 2 * NQ:2 + 2 * NQ])
            for (r0, r1) in OUTSLICES:
                nc.sync.dma_start(out[r0:r1, c_off:c_off + W], y16[r0:r1, :])
            c_off += W

    nc.compile()
    _PROGRAM = nc
    return nc


LAST_EXEC_NS = None


def kernel(**inputs) -> np.ndarray:
    global LAST_EXEC_NS
    import os
    from concourse.bass_utils import run_bass_kernel_spmd

    Wm, consts = _fit_tables(inputs)
    bf16 = _bf16_dtype()
    X = np.ascontiguousarray(
        np.asarray(inputs["wave_features"], np.float32).reshape(B * S, NW)
    )

    in_maps = []
    for k in range(NCORES):
        XT = X[k * RPC:(k + 1) * RPC].T  # [118, 16384], col = r*F + f
        XR = XT.reshape(NW, NREP, F)
        # chunk-concatenated layout: per chunk j, cols = r-major blocks
        blocks = []
        c_off = 0
        for W in CHUNKS:
            blocks.append(XR[:, :, c_off:c_off + W].reshape(NW, NREP * W))
            c_off += W
        xt_k = np.ascontiguousarray(
            np.concatenate(blocks, axis=1).astype(bf16))
        in_maps.append({"xt": xt_k, "wm": Wm, "cst": consts})

    nc = _build_program()
    trace = os.environ.get("BASS_KERNEL_PROFILE") == "1"
    res = run_bass_kernel_spmd(nc, in_maps, core_ids=list(range(NCORES)),
                               trace=trace)
    LAST_EXEC_NS = res.exec_time_ns
    # unshard: [104, 2048] fp16 feature-major -> [16384, 13] f32 row-major
    outs = []
    for k in range(NCORES):
        buf = np.asarray(res.results[k]["out"], np.float16).astype(np.float32)
        outs.append(buf.reshape(NREP, NB, F).transpose(0, 2, 1).reshape(RPC, NB))
    return np.concatenate(outs, axis=0).reshape(B, S, NB)


# revision 5
# speedup vs baseline: 2.0273x; 1.2618x over previous
"""Trainium2 Bass kernel for nn_CelestialWaveAggregator.

Math: out[b,s,c] = tanh(h_c(agg[b,s,c])) where agg = wave_features @ M.T (M is
the per-body softmax aggregation matrix over ragged wave groups) and h_c is the
per-body 1->32->64->32->1 gelu MLP collapsed to a *univariate* function of the
aggregated scalar.

v4 device strategy (8 cores, batch-sharded 2048*8 rows/core):
  - fp16 inputs: half the DMA bytes of f32.  Input DMA is issued as 4 row-
    slice dma_starts per chunk spread over the four HWDGE queues (sync,
    scalar, vector, tensor) — HWDGE has ~70ns/descriptor overhead vs SWDGE's
    ~400ns, and >=8 concurrent dma_start instructions are needed to engage
    all 16 DMA engines.  All chunk tiles stay resident (no buffer reuse), so
    the hoisted DMA issues never block an engine.
  - PE: agg matmuls accumulate [104, 512] PSUM tiles from 8 replica-masked
    fp16 weight blocks (8 row-chunks x 13 bodies on partitions).
  - Polynomial: degree-8, factored into 4 quadratics
    p(t) = a_D * prod_i (t^2 + alpha_i t + beta_i).  Two quadratics are
    computed as (t+g)^2 + d with the square on the ACT engine; the other two
    as tensor_scalar (4x fp16 mode) + tensor_tensor add (2x) on DVE.  The
    product tree is 3 tensor_tensor mults.  scalar_tensor_tensor is avoided
    (it has NO fp16 fast mode).
  - ACT also produces t = Identity(ps + bias) in fp16 and the final
    tanh(a_D * prod) with per-partition scale.
Output is stored feature-major [104, 2048] fp16 (2 row-slice stores per chunk
on the sync queue); host permutes to row-major f32 on gather.  Quadratics are
ordered so fp16 partial products stay in range.  Host-simulated end-to-end rel
err ~8.8e-3 (gate 2e-2).
"""

import math

import numpy as np

# ---- problem constants (hardcoded per contract) ----
LENS = np.array([9, 9, 9, 9, 9, 9, 9, 9, 9, 9, 12, 8, 3])
STARTS = np.concatenate([[5], 5 + np.cumsum(LENS)[:-1]])
MAXW, NW, NB = 12, 118, 13
B, S = 32, 4096
NCORES = 8
RPC = (B * S) // NCORES          # 16384 rows per core
NREP = 8                         # replica groups on partitions (8*13=104)
NP_USED = NREP * NB              # 104 used partitions
F = RPC // NREP                  # 2048 free columns per partition (exact)
CHUNKS = [512, 512, 512, 512]    # pipeline chunk widths (sum = F)
DEG = 8                          # polynomial degree (even)
NQ = DEG // 2                    # quadratic factors
NACT = 2                         # quadratics evaluated via ACT Square
NCONST = 2 + 2 * NQ              # nmid, alphas/gammas, betas/deltas, aD
ROWSLICES = [(0, 30), (30, 60), (60, 90), (90, 118)]   # input DMA row split
OUTSLICES = [(0, 52), (52, 104)]                       # output DMA row split

_f64 = np.float64


def _erf(x):
    try:
        from scipy.special import erf
        return erf(x)
    except Exception:
        return np.vectorize(math.erf)(x)


def _gelu(x):
    return 0.5 * x * (1.0 + _erf(x / np.sqrt(2.0)))


def _build_M(agg_logits):
    """Dense [13, 118] aggregation matrix from ragged softmax groups."""
    al = np.asarray(agg_logits, _f64)
    valid = np.arange(MAXW)[None, :] < LENS[:, None]
    logits = np.where(valid, al, -np.inf)
    w = np.exp(logits - logits.max(axis=-1, keepdims=True))
    w = w / w.sum(axis=-1, keepdims=True)
    w = np.where(valid, w, 0.0)
    M = np.zeros((NB, NW))
    idx = np.clip(STARTS[:, None] + np.arange(MAXW)[None, :], 0, NW - 1)
    for c in range(NB):
        for j in range(MAXW):
            M[c, idx[c, j]] += w[c, j]
    return M


def _h_fn(x, c, W1, b1, W2, b2, W3, b3, W4, b4):
    """Pre-tanh univariate MLP for body c, float64."""
    a = x[:, None] * W1[c, 0][None, :] + b1[c]
    h1 = _gelu(a)
    h2 = _gelu(h1 @ W2[c] + b2[c])
    h3 = _gelu(h2 @ W3[c] + b3[c])
    return h3 @ W4[c][:, 0] + b4[c, 0]


def _fit_tables(inputs):
    """Host precompute: aggregation matrix, per-body quadratic factorizations.

    Returns (Wm fp16 [118, 8*104], consts f32 [104, NCONST]).
    consts cols: 0 = -mid*invhalf.
    For i < NACT (ACT-square path, q_i = (t+g_i)^2 + d_i):
        col 1+i = g_i, col 1+NQ+i = d_i.
    For i >= NACT (DVE path, q_i = u + a_i t + b_i):
        col 1+i = alpha_i, col 1+NQ+i = beta_i.
    col 1+2NQ = a_D (leading coeff, applied as tanh scale)."""
    M = _build_M(inputs["agg_logits"])
    Wmlp = {k: np.asarray(inputs[k], _f64) for k in
            ("W1", "b1", "W2", "b2", "W3", "b3", "W4", "b4")}

    # calibration: per-body agg range from the actual data (+ margin)
    X = np.asarray(inputs["wave_features"], np.float32).reshape(-1, NW)
    agg = X.astype(_f64) @ M.T
    lo = agg.min(axis=0)
    hi = agg.max(axis=0)
    m = 0.12 * (hi - lo)
    lo, hi = lo - m, hi + m
    mid = 0.5 * (lo + hi)
    invhalf = 2.0 / (hi - lo)

    quads = np.zeros((NB, NQ, 2))
    aD = np.zeros(NB)
    tg2 = np.linspace(-1, 1, 513)
    for c in range(NB):
        xs = np.linspace(lo[c], hi[c], 3001)
        hs = _h_fn(xs, c, **Wmlp)
        ys = np.tanh(hs)
        t = (xs - mid[c]) * invhalf[c]
        V = np.polynomial.chebyshev.chebvander(t, DEG)
        wgt = 1.0 / np.cosh(hs) ** 2 + 1e-4
        for _ in range(10):
            sw = np.sqrt(wgt)
            coef, *_r = np.linalg.lstsq(V * sw[:, None], hs * sw, rcond=None)
            err = np.abs(np.tanh(V @ coef) - ys)
            wgt = wgt * (1.0 + 1.5 * err / (err.max() + 1e-12))
        a = np.polynomial.chebyshev.cheb2poly(coef)     # power basis a[0..DEG]
        aD[c] = a[DEG]
        roots = np.roots(a[::-1])
        # pair complex-conjugate roots; pair real roots sorted-adjacent
        cplx = [r for r in roots if abs(r.imag) > 1e-9]
        real = sorted(r.real for r in roots if abs(r.imag) <= 1e-9)
        qs = []
        used = set()
        for i, r in enumerate(cplx):
            if i in used:
                continue
            for j2 in range(i + 1, len(cplx)):
                if j2 not in used and abs(cplx[j2].conjugate() - r) < \
                        1e-6 * max(1.0, abs(r)):
                    used.add(i)
                    used.add(j2)
                    qs.append((-2 * r.real, abs(r) ** 2))
                    break
        for i in range(0, len(real), 2):
            qs.append((-(real[i] + real[i + 1]), real[i] * real[i + 1]))
        assert len(qs) == NQ, f"body {c}: got {len(qs)} quadratic factors"
        # interleave large/small |q|max so fp16 partial products stay in range
        mx = [np.abs(tg2 ** 2 + al * tg2 + be).max() for al, be in qs]
        order = list(np.argsort(mx)[::-1])
        inter = []
        lo_i, hi_i = 0, len(order) - 1
        for k2 in range(len(order)):
            inter.append(order[hi_i] if k2 % 2 == 0 else order[lo_i])
            if k2 % 2 == 0:
                hi_i -= 1
            else:
                lo_i += 1
        qs = [qs[i] for i in inter]
        quads[c] = qs
        P = np.ones_like(tg2)
        for al, be in qs:
            P = P * (tg2 ** 2 + al * tg2 + be)
            assert np.abs(P).max() < 3e4, f"body {c}: fp16 overflow risk"

    # device constant tensors
    Wm = np.zeros((NW, NREP * NP_USED), np.float16)
    Ms = (M * invhalf[:, None]).T.astype(np.float32)  # [118, 13]
    for r in range(NREP):
        for c in range(NB):
            Wm[:, r * NP_USED + r * NB + c] = Ms[:, c].astype(np.float16)
    consts = np.zeros((NP_USED, NCONST), np.float32)
    for r in range(NREP):
        for c in range(NB):
            q = r * NB + c
            consts[q, 0] = -mid[c] * invhalf[c]
            for i in range(NQ):
                al, be = quads[c, i]
                if i < NACT:
                    # ACT path: (t + g)^2 + d
                    consts[q, 1 + i] = al / 2.0
                    consts[q, 1 + NQ + i] = be - al * al / 4.0
                else:
                    consts[q, 1 + i] = al
                    consts[q, 1 + NQ + i] = be
            consts[q, 1 + 2 * NQ] = aD[c]
    return Wm, consts


def _split512(w):
    """Split a chunk width into PSUM-bank-aligned matmul runs (<=512 each)."""
    out = []
    while w > 0:
        out.append(min(w, 512))
        w -= out[-1]
    return out


_PROGRAM = None


def _build_program():
    """Build + compile the (SPMD, per-core) Bass/Tile program once."""
    global _PROGRAM
    if _PROGRAM is not None:
        return _PROGRAM

    from contextlib import ExitStack
    import concourse.bacc as bacc
    import concourse.tile as tile
    import concourse.mybir as mybir
    from concourse._compat import axon_active

    f32 = mybir.dt.float32
    f16 = mybir.dt.float16
    Alu = mybir.AluOpType
    Act = mybir.ActivationFunctionType

    nc = bacc.Bacc(
        "TRN2",
        target_bir_lowering=False,
        debug=not axon_active(),
        enable_asserts=True,
        num_devices=NCORES,
    )
    xt = nc.dram_tensor("xt", [NW, RPC], f16, kind="ExternalInput").ap()
    wm = nc.dram_tensor("wm", [NW, NREP * NP_USED], f16,
                        kind="ExternalInput").ap()
    cst = nc.dram_tensor("cst", [NP_USED, NCONST], f32, kind="ExternalInput").ap()
    out = nc.dram_tensor("out", [NP_USED, F], f16, kind="ExternalOutput").ap()

    with tile.TileContext(nc) as tc, ExitStack() as ctx:
        cpool = ctx.enter_context(tc.tile_pool(name="consts", bufs=1))
        xpool = ctx.enter_context(tc.tile_pool(name="xin", bufs=len(CHUNKS)))
        ppool = ctx.enter_context(tc.tile_pool(name="ps", bufs=2, space="PSUM"))
        tpool = ctx.enter_context(tc.tile_pool(name="tt", bufs=2))
        upool = ctx.enter_context(tc.tile_pool(name="uu", bufs=2))
        qpool = ctx.enter_context(tc.tile_pool(name="qq", bufs=2 * NQ))
        ypool = ctx.enter_context(tc.tile_pool(name="yy", bufs=2))

        wm_sb = cpool.tile([NW, NREP * NP_USED], f16)
        nc.sync.dma_start(wm_sb[:], wm[:])
        cst_sb = cpool.tile([NP_USED, NCONST], f32)
        nc.sync.dma_start(cst_sb[:], cst[:])

        # hoisted input loads: 4 row-slices per chunk, alternating between the
        # two HWDGE queues (sync/SP and scalar/Activation)
        dma_engines = [nc.sync, nc.scalar, nc.sync, nc.scalar]
        xcs = []
        c_off = 0
        for j, W in enumerate(CHUNKS):
            SUP = NREP * W
            base = NREP * c_off
            xc = xpool.tile([NW, SUP], f16, tag=f"xin{j}")
            for si, (r0, r1) in enumerate(ROWSLICES):
                dma_engines[si].dma_start(
                    xc[r0:r1, :], xt[r0:r1, base:base + SUP])
            xcs.append(xc)
            c_off += W

        c_off = 0
        for j, W in enumerate(CHUNKS):
            xc = xcs[j]
            ps = ppool.tile([NP_USED, W], f32, tag="ps")
            for r in range(NREP):
                h0 = 0
                for hw in _split512(W):
                    nc.tensor.matmul(
                        ps[:, h0:h0 + hw],
                        wm_sb[:, r * NP_USED:(r + 1) * NP_USED],
                        xc[:, r * W + h0: r * W + h0 + hw],
                        start=(r == 0),
                        stop=(r == NREP - 1),
                    )
                    h0 += hw
            # t = agg*invhalf - mid*invhalf, fp16; u = t^2
            t16 = tpool.tile([NP_USED, W], f16, tag="tt")
            nc.scalar.activation(t16[:], ps[:], Act.Identity,
                                 bias=cst_sb[:, 0:1])
            u16 = upool.tile([NP_USED, W], f16, tag="uu")
            nc.scalar.activation(u16[:], t16[:], Act.Square)
            qs = []
            for i in range(NQ):
                q = qpool.tile([NP_USED, W], f16, tag=f"q{i}")
                if i < NACT:
                    # q_i = Square(t + g_i) + d_i : square on ACT, add on DVE
                    nc.scalar.activation(q[:], t16[:], Act.Square,
                                         bias=cst_sb[:, 1 + i:2 + i])
                    nc.vector.tensor_scalar(
                        q[:], q[:], cst_sb[:, 1 + NQ + i:2 + NQ + i], None,
                        op0=Alu.add)
                else:
                    # q_i = u + alpha_i t + beta_i : all on DVE
                    nc.vector.tensor_scalar(
                        q[:], t16[:], cst_sb[:, 1 + i:2 + i],
                        cst_sb[:, 1 + NQ + i:2 + NQ + i],
                        op0=Alu.mult, op1=Alu.add)
                    nc.vector.tensor_tensor(q[:], q[:], u16[:], op=Alu.add)
                qs.append(q)
            # product tree (in-place into q0/q2)
            nc.vector.tensor_tensor(qs[0][:], qs[0][:], qs[1][:], op=Alu.mult)
            nc.vector.tensor_tensor(qs[2][:], qs[2][:], qs[3][:], op=Alu.mult)
            nc.vector.tensor_tensor(qs[0][:], qs[0][:], qs[2][:], op=Alu.mult)
            # y = tanh(a_D * prod)
            y16 = ypool.tile([NP_USED, W], f16, tag="yy")
            nc.scalar.activation(y16[:], qs[0][:], Act.Tanh,
                                 scale=cst_sb[:, 1 + 2 * NQ:2 + 2 * NQ])
            for (r0, r1) in OUTSLICES:
                nc.sync.dma_start(out[r0:r1, c_off:c_off + W], y16[r0:r1, :])
            c_off += W

    nc.compile()
    _PROGRAM = nc
    return nc


LAST_EXEC_NS = None


def kernel(**inputs) -> np.ndarray:
    global LAST_EXEC_NS
    import os
    from concourse.bass_utils import run_bass_kernel_spmd

    Wm, consts = _fit_tables(inputs)
    X = np.ascontiguousarray(
        np.asarray(inputs["wave_features"], np.float32).reshape(B * S, NW)
    )

    in_maps = []
    for k in range(NCORES):
        XT = X[k * RPC:(k + 1) * RPC].T  # [118, 16384], col = r*F + f
        XR = XT.reshape(NW, NREP, F)
        # chunk-concatenated layout: per chunk j, cols = r-major blocks
        blocks = []
        c_off = 0
        for W in CHUNKS:
            blocks.append(XR[:, :, c_off:c_off + W].reshape(NW, NREP * W))
            c_off += W
        xt_k = np.ascontiguousarray(
            np.concatenate(blocks, axis=1).astype(np.float16))
        in_maps.append({"xt": xt_k, "wm": Wm, "cst": consts})

    nc = _build_program()
    trace = os.environ.get("BASS_KERNEL_PROFILE") == "1"
    res = run_bass_kernel_spmd(nc, in_maps, core_ids=list(range(NCORES)),
                               trace=trace)
    LAST_EXEC_NS = res.exec_time_ns
    # unshard: [104, 2048] fp16 feature-major -> [16384, 13] f32 row-major
    outs = []
    for k in range(NCORES):
        buf = np.asarray(res.results[k]["out"], np.float16).astype(np.float32)
        outs.append(buf.reshape(NREP, NB, F).transpose(0, 2, 1).reshape(RPC, NB))
    return np.concatenate(outs, axis=0).reshape(B, S, NB)
